# revision 3
# baseline (speedup 1.0000x reference)
"""3-level Haar DWT feature kernel for Trainium2 (8 NeuronCores, data-parallel).

Full input x: [256, 131072] f32. Output: [256, 131072] f32 =
concat([cA3, cD3, cD2, cD1], axis=1) per row (pywt wavedec order).

Sharding: batch dim split 8 ways (32 rows per core), no cross-core comm.

Per-core structure (measured ~92us/core vs ~87us for a bare HBM
load+store copy of the same bytes — i.e. at the memory roofline):

- 2 chunks of 16 rows, each one [128, 16384] SBUF tile: partition
  p = r_local*8 + p_sub holds row elements [p_sub*16384, (p_sub+1)*16384).
  Haar pairs (2k, 2k+1) are adjacent along the free dim at every level.
- Loads are SWDGE (gpsimd ring) DMAs with an inline f32->bf16 cast: HBM
  reads stay f32 (16MiB/core) but SBUF tiles halve, which is what lets a
  16-row chunk double-buffer inside SBUF. Each chunk loads in 4 slices so
  DVE starts after ~1MiB instead of 8MiB.
- All intermediates are bf16 (tolerance is 2e-2; measured rel err 3.3e-3).
  DVE does the pairwise sum/diff (tensor_tensor) and the 1/sqrt(2)^l
  scales (tensor_scalar_mul, in place) per slice; ScalarE/PE stay idle.
- Stores are SWDGE DMAs with an inline bf16->f32 cast, one per output
  segment per chunk (d1 split in 2): 10 stores total. Big chunks matter:
  descriptor count per chunk is fixed at 128/segment, and per-DMA-op cost
  (~2-5us, HBM write-receipt bound) made finer chunkings (4x8-row chunks
  = 16+ stores) 25-45us slower. Single-ring SWDGE for both directions
  saturates the ~360GB/s per-core HBM cap; separate HWDGE rings gave no
  additional speedup once op count was minimized.

reps/loop_n exist for the differential timing harness (bench2/tune):
loop_n wraps the body in a For_i hardware loop so NEFF size is
independent of trip count and host dispatch cost cancels in a slope.
"""

import numpy as np

import concourse.bacc as bacc
import concourse.bass as bass
import concourse.mybir as mybir
from concourse.tile import TileContext
from concourse.bass_utils import run_bass_kernel_spmd

INV_SQRT2 = 0.7071067811865476
C1 = INV_SQRT2          # cD1 scale
C2 = 0.5                # cD2 scale
C3 = 0.5 * INV_SQRT2    # cA3 / cD3 scale

N_CORES = 8
B, L = 256, 131072
ROWS = B // N_CORES     # 32 rows per core

R = 16                  # rows per chunk
P_SUB = 128 // R        # 8 partitions per row
F = (L * R) // 128      # 16384 free elems per partition
N_GROUPS = ROWS // R    # 2 chunks
SPLITS = 4              # load/compute slices per chunk
D1_STORE_SPLITS = 2

FP32 = mybir.dt.float32
BF16 = mybir.dt.bfloat16

# kept for bench2 compatibility (R=8 legacy layout constants unused)
KERNEL_VERSION = "v52"


def _pairs(ap):
    """[128, N] AP -> (even, odd) stride-2 APs of shape [128, N//2]."""
    p3 = ap.rearrange("p (n two) -> p n two", two=2)
    return p3[:, :, 0], p3[:, :, 1]


def _build_bass(reps=1, loop_n=None):
    nc = bacc.Bacc(
        "TRN2",
        target_bir_lowering=False,
        debug=False,
        num_devices=N_CORES,
    )
    x = nc.dram_tensor("x", [ROWS, L], FP32, kind="ExternalInput")
    out = nc.dram_tensor("out", [ROWS, L], FP32, kind="ExternalOutput")
    sub, add = mybir.AluOpType.subtract, mybir.AluOpType.add

    with TileContext(nc) as tc:
        with (
            tc.tile_pool(name="xin", bufs=2) as xin_pool,
            tc.tile_pool(name="mid", bufs=1) as mid_pool,
        ):
            def body():
                for g in range(N_GROUPS):
                    rows = slice(g * R, (g + 1) * R)
                    xt = xin_pool.tile([128, F], BF16, tag="xt")
                    du1 = mid_pool.tile([128, F // 2], BF16, tag="du1")
                    s1 = mid_pool.tile([128, F // 2], BF16, tag="s1")
                    du2 = mid_pool.tile([128, F // 4], BF16, tag="du2")
                    s2 = mid_pool.tile([128, F // 4], BF16, tag="s2")
                    du3 = mid_pool.tile([128, F // 8], BF16, tag="du3")
                    su3 = mid_pool.tile([128, F // 8], BF16, tag="su3")

                    xr = x[rows].rearrange("r (p f) -> (r p) f", p=P_SUB)
                    for sidx in range(SPLITS):
                        W = F // SPLITS
                        lo = sidx * W
                        sl = slice(lo, lo + W)
                        # SWDGE load, inline f32 -> bf16 cast
                        nc.gpsimd.dma_start(out=xt[:, sl], in_=xr[:, sl])

                        ev, od = _pairs(xt[:, sl])
                        h1 = slice(lo // 2, (lo + W) // 2)
                        nc.vector.tensor_tensor(out=du1[:, h1], in0=ev,
                                                in1=od, op=sub)
                        nc.vector.tensor_scalar_mul(du1[:, h1], du1[:, h1],
                                                    C1)
                        nc.vector.tensor_tensor(out=s1[:, h1], in0=ev,
                                                in1=od, op=add)
                        ev, od = _pairs(s1[:, h1])
                        h2 = slice(lo // 4, (lo + W) // 4)
                        nc.vector.tensor_tensor(out=du2[:, h2], in0=ev,
                                                in1=od, op=sub)
                        nc.vector.tensor_scalar_mul(du2[:, h2], du2[:, h2],
                                                    C2)
                        nc.vector.tensor_tensor(out=s2[:, h2], in0=ev,
                                                in1=od, op=add)
                        ev, od = _pairs(s2[:, h2])
                        h3 = slice(lo // 8, (lo + W) // 8)
                        nc.vector.tensor_tensor(out=du3[:, h3], in0=ev,
                                                in1=od, op=sub)
                        nc.vector.tensor_scalar_mul(du3[:, h3], du3[:, h3],
                                                    C3)
                        nc.vector.tensor_tensor(out=su3[:, h3], in0=ev,
                                                in1=od, op=add)
                        nc.vector.tensor_scalar_mul(su3[:, h3], su3[:, h3],
                                                    C3)

                    # SWDGE stores, inline bf16 -> f32 cast.
                    # Row segments: [cA3 | cD3 | cD2 | cD1]
                    def gstore(tile, seg_lo, seg_hi, ssplits=1):
                        n = tile.shape[-1]
                        seg = out[rows, seg_lo:seg_hi].rearrange(
                            "r (p f) -> r p f", p=P_SUB)
                        for s in range(ssplits):
                            a = s * n // ssplits
                            b = (s + 1) * n // ssplits
                            nc.gpsimd.dma_start(out=seg[:, :, a:b],
                                                in_=tile[:, a:b])

                    gstore(du1, L // 2, L, D1_STORE_SPLITS)
                    gstore(du2, L // 4, L // 2)
                    gstore(du3, L // 8, L // 4)
                    gstore(su3, 0, L // 8)

            if loop_n is None:
                for _ in range(reps):
                    body()
            else:
                with tc.For_i(0, loop_n, 1):
                    body()
    nc.compile()
    return nc


_NC_CACHE = None


def _get_nc():
    global _NC_CACHE
    if _NC_CACHE is None:
        _NC_CACHE = _build_bass()
    return _NC_CACHE


def run_sharded(x, **kwargs):
    """Run on 8 cores; returns (full_output, BassKernelResults)."""
    x = np.ascontiguousarray(np.asarray(x), dtype=np.float32)
    assert x.shape == (B, L), x.shape
    nc = _get_nc()
    in_maps = [
        {"x": np.ascontiguousarray(x[i * ROWS:(i + 1) * ROWS])}
        for i in range(N_CORES)
    ]
    res = run_bass_kernel_spmd(nc, in_maps, list(range(N_CORES)), **kwargs)
    full = np.concatenate([res.results[i]["out"] for i in range(N_CORES)], axis=0)
    return full, res


def kernel(x):
    out, _ = run_sharded(x)
    return out
